# revision 18
# baseline (speedup 1.0000x reference)
"""Trainium2 Bass kernel for the BackwardVariableSplitter pair scorer.

reference math:
    context = relu(nse @ Wc + bc)                      # [128]
    queries = pve @ Wq + bq + context                  # [1024, 128]
    keys    = pve @ Wk + bk + context                  # [1024, 128]
    q_proj  = queries @ W1[:128]                       # [1024, 128]
    k_proj  = keys @ W1[128:]                          # [1024, 128]
    hidden[i,j] = relu(q_proj[i] + k_proj[j] + b1)     # [1024, 1024, 128]
    scores[i,j] = hidden[i,j] @ W2 + b2                # [1024, 1024]
    out = scores[i, j] for i < j, row-major            # [523776]

The O(n*d*h) projections are tiny and run on the host; the O(n^2*h)
relu + weighted-reduce runs on 8 NeuronCores.

Sharding: core d owns query rows {i : i % 8 == d} (interleaved), so the
triangular (j > i) workload is balanced and the SPMD program is identical
on every core: local row k (global i = 8k + d) computes columns
j in [8k, 1024).

Per-core device program (measured engine costs: DVE tensor_scalar fp16 4x
~129 + 0.26*w ns; ACT activation ~186 + 0.833*w ns; DVE tensor_tensor
fp16 2x ~0.52*w per row):

  - DVE per-row (rows 0..N_D): X = relu(kpt[:, 8k:] + qb_k) via one
    tensor_scalar (add, max0) at 4x perf mode. Widest rows go here - the
    129ns fixed cost amortizes best over wide rows.
  - ACT per-row (rows N_D..96 + a small warmup batch >= 96): same via
    activation(Relu, bias=qb_k).
  - DVE mega (narrow rows >= 96, rectangles of 8 rows): uses the exact
    identity relu(a+b) = max(a,-b) + b, so a single stock tensor_tensor
    MAX instruction computes X' = max(kpt[:, j0:], -qb_k) for 8 rows at
    once. The broadcast of -qb_k along j is expressed with a
    duplicated-pair buffer (mqb2[:, 2k] = mqb2[:, 2k+1] = -qb[:, k]) and
    a [k][w/2][pair] access pattern whose innermost dim has stride 1 -
    keeping every operand eligible for the DVE 2x_1P perf mode. The
    missing "+ qb" term is linear, so it folds into the host-side
    per-row constant C_k = w2 . qb_k added after the PE reduce.
  - TensorE: one-hot W2 stationary window [128, 32] puts w2 in column
    (k//4), so row k's scores accumulate into PSUM partition
    32*(k%4) + k//4. tile_position=(0, 32*(k%4)) spreads consecutive
    rows across the 4 PE column groups (concurrent streams). Banks are
    pre-zeroed by start=True all-zero matmuls.
  - PSUM bank A = columns [0,512), bank B = [512,1024). B is evicted in
    two halves (B1 [512,768) whose writers are rows < 96, B2 [768,1024))
    so output DMA overlaps the narrow-row tail.
  - Inputs ship as one 1216B/partition "head" DMA (qb fp32 + -qb fp16
    pairs + w2 window) plus two kpt chunks, sized >= 1KB/partition lines
    for DMA bandwidth. Output is fp16 (host upcasts); host adds C_k + b2.
"""

import os
import numpy as np

N = 1024
E = 256
H = 128
NCORES = 8
NROWS = N // NCORES  # 128 local rows per core

# engine split (tunable): rows [0, N_D) DVE per-row; [N_D, 96) ACT per-row;
# ACT_WARM rows starting at 96 also ACT (they only need the first kpt chunk,
# so ACT can start before kpt[512:768] lands); the rest mega-TT on DVE.
# DP-optimal block assignment (measured costs): DVE per-row on blocks
# 0-5 and 7, ACT on blocks 6 and 8-11, DVE-mega on blocks 12-15.
DVE_ROWS = list(range(0, 48)) + list(range(56, 65))
ACT_ROWS = list(range(48, 56)) + list(range(65, 96))
MEGA0 = 96
RECT = 8  # rows per mega rectangle

_PROG_CACHE = {}


def psum_partition(k: int) -> int:
    return 32 * (k % 4) + k // 4


def _mega_rects():
    """[(k0, nrows, j0, w0), ...] covering rows [MEGA0, 128)."""
    rects = []
    k = MEGA0
    while k < NROWS:
        nr = min(RECT, NROWS - k)
        j0 = 8 * k
        rects.append((k, nr, j0, N - j0))
        k += nr
    return rects  # [(96,8,768,256), (104,8,832,192), (112,8,896,128), (120,8,960,64)]


def _build_program():
    import concourse.bacc as bacc
    import concourse.tile as tile
    import concourse.mybir as mybir

    nc = bacc.Bacc(
        "TRN2",
        target_bir_lowering=False,
        enable_partition_id=False,
        detect_race_conditions=False,
    )

    fp16 = mybir.dt.float16
    fp32 = mybir.dt.float32
    u8 = mybir.dt.uint8

    HEADB = 512 + 192 + 512  # qbt fp32 | w2w fp16 | mqb2 fp16 (bytes/partition)
    head_d = nc.dram_tensor("head", [H, HEADB], u8, kind="ExternalInput")
    kpt_d = nc.dram_tensor("kpt", [H, N], fp16, kind="ExternalInput")
    out_d = nc.dram_tensor("out", [H, N], fp16, kind="ExternalOutput")

    rects = _mega_rects()

    with tile.TileContext(nc) as tc:
        with (
            tc.tile_pool(name="sb", bufs=1) as sb,
            tc.tile_pool(name="ps", bufs=1, space="PSUM") as ps,
        ):
            head = sb.tile([H, HEADB], u8)
            kpt = sb.tile([H, N], fp16)
            zw = sb.tile([H, 256], fp16)
            out_sb = sb.tile([H, N], fp16)

            qbt = head[:, 0:512].bitcast(fp32)     # [H, 128] fp32 (+qb)
            w2w = head[:, 512:704].bitcast(fp16)   # [H, 96]
            mqb2 = head[:, 704:1216].bitcast(fp16)  # [H, 256] fp16 (-qb pairs)

            # few, big input DMAs (extra DMAs cost ~0.5-1us of queue
            # serialization each); the kpt tail ships first since all the
            # early (narrow) work reads only columns >= 640.
            nc.sync.dma_start(head[:], head_d[:, :])
            nc.scalar.dma_start(kpt[:, 640:N], kpt_d[:, 640:N])
            nc.scalar.dma_start(kpt[:, 0:640], kpt_d[:, 0:640])

            nc.gpsimd.memset(zw[:], 0.0)
            # warm up ACT's Relu table while the DMAs stream
            nc.scalar.activation(
                out_sb[:, 0:16], out_sb[:, 0:16],
                mybir.ActivationFunctionType.Relu,
            )

            psA = ps.tile([H, 512], fp32)  # columns [0, 512)
            psB = ps.tile([H, 512], fp32)  # columns [512, 1024)

            for bank in (psA, psB):
                for half in range(2):
                    nc.tensor.matmul(
                        bank[:, 256 * half: 256 * half + 256],
                        zw[:, 0:H],
                        zw[:],
                        start=True,
                        stop=False,
                        skip_group_check=True,
                    )

            x_dve = {}
            x_act = {}
            x_rect = {}
            for k in DVE_ROWS:
                x_dve[k] = sb.tile([H, N - 8 * k], fp16, name=f"xd{k}")
            for k in ACT_ROWS:
                x_act[k] = sb.tile([H, N - 8 * k], fp16, name=f"xa{k}")
            for (k0, nr, j0, w0) in rects:
                x_rect[k0] = sb.tile([H, nr, w0], fp16, name=f"xr{k0}")

            def emit_mms(k, xap, js, stop_a=False, stop_b=False):
                """PE reduce for local row k whose X tile starts at col js."""
                g = k % 4
                m = k // 4
                lhsT = w2w[:, 63 - m: 95 - m]
                pslice = slice(32 * g, 32 * g + 32)
                if js < 512:
                    wa = 512 - js
                    nc.tensor.matmul(
                        psA[pslice, js:512], lhsT, xap[:, 0:wa],
                        start=False, stop=stop_a, skip_group_check=True,
                        tile_position=(0, 32 * g),
                    )
                    nc.tensor.matmul(
                        psB[pslice, :], lhsT, xap[:, wa: wa + 512],
                        start=False, stop=stop_b, skip_group_check=True,
                        tile_position=(0, 32 * g),
                    )
                else:
                    nc.tensor.matmul(
                        psB[pslice, js - 512: 512], lhsT, xap[:],
                        start=False, stop=stop_b, skip_group_check=True,
                        tile_position=(0, 32 * g),
                    )

            def dve_row(k, stop_a=False, stop_b=False):
                x = x_dve[k][:, :]
                nc.vector.tensor_scalar(
                    x, kpt[:, 8 * k: N], qbt[:, k: k + 1], 0.0,
                    op0=mybir.AluOpType.add, op1=mybir.AluOpType.max,
                )
                emit_mms(k, x, 8 * k, stop_a=stop_a, stop_b=stop_b)

            def act_row(k, stop_a=False, stop_b=False):
                x = x_act[k][:, :]
                nc.scalar.activation(
                    x, kpt[:, 8 * k: N],
                    mybir.ActivationFunctionType.Relu,
                    bias=qbt[:, k: k + 1], scale=1.0,
                )
                emit_mms(k, x, 8 * k, stop_a=stop_a, stop_b=stop_b)

            def mega_rect(rect, stop_a=False, stop_b=False):
                k0, nr, j0, w0 = rect
                xt = x_rect[k0]
                out4 = xt[:, :, :].rearrange("p k (a t) -> p k a t", t=2)
                in0 = (
                    kpt[:, None, j0: j0 + w0]
                    .broadcast_to([H, nr, w0])
                    .rearrange("p k (a t) -> p k a t", t=2)
                )
                in1 = (
                    mqb2[:, 2 * k0: 2 * (k0 + nr)]
                    .rearrange("p (k t) -> p k t", t=2)[:, :, None, :]
                    .broadcast_to([H, nr, w0 // 2, 2])
                )
                nc.vector.tensor_max(out4, in0, in1)
                for t in range(nr):
                    k = k0 + t
                    emit_mms(
                        k, xt[:, t, :], j0,
                        stop_b=(stop_b and t == nr - 1),
                    )

            # ---- schedule ----
            # Per-engine unit order is chosen for data arrival (narrow work
            # first: it only needs the kpt tail + head DMAs) and for early
            # PSUM-region completion. Units from the two engines are EMITTED
            # in predicted-completion order so the TensorE FIFO sees matmuls
            # in the order their rhs tiles actually materialize.
            def cost(unit):
                kind, arg = unit
                if kind == "rect":
                    k0, nr, j0, w0 = arg
                    return 129 + 0.521 * nr * w0
                if kind == "drow":
                    return 129 + 0.26 * (N - 8 * arg)
                if kind == "arow":
                    return 186 + 0.833 * (N - 8 * arg)
                return 500.0  # evictions

            def ready(unit):
                kind, arg = unit
                if kind == "drow":
                    return 1300.0  # kpt[0:512] chunk
                if kind == "arow" and arg < 64:
                    return 1300.0
                if kind == "arow":
                    return 600.0  # kpt[512:1024] chunk
                return 0.0

            # stage 1 contains every psA and psB1 writer; stage 2 is the
            # pure-B2 mega rect. Narrow work leads (it only needs the kpt
            # tail chunk + head DMAs).
            dve1 = (
                [("rect", rects[2]), ("rect", rects[1]), ("rect", rects[0])]
                + [("drow", k) for k in DVE_ROWS]
            )
            dve2 = [("rect", rects[3])]
            act1 = (
                [("arow", k) for k in range(95, 87, -1)]
                + [("arow", k) for k in range(48, 56)]
                + [("arow", k) for k in range(87, 64, -1)]
            )
            act2 = []

            def emit(unit, stop_a=False, stop_b=False):
                kind, arg = unit
                if kind == "rect":
                    mega_rect(arg, stop_a=stop_a, stop_b=stop_b)
                elif kind == "drow":
                    dve_row(arg, stop_a=stop_a, stop_b=stop_b)
                elif kind == "arow":
                    act_row(arg, stop_a=stop_a, stop_b=stop_b)
                elif kind == "evA":
                    nc.vector.tensor_copy(out_sb[:, 0:512], psA[:])
                    nc.sync.dma_start(out_d[:, 0:512], out_sb[:, 0:512])
                elif kind == "evB1":
                    nc.scalar.copy(out_sb[:, 512:768], psB[:, 0:256])
                    nc.scalar.dma_start(out_d[:, 512:768], out_sb[:, 512:768])
                elif kind == "evB2":
                    nc.vector.tensor_copy(out_sb[:, 768:N], psB[:, 256:512])
                    nc.sync.dma_start(out_d[:, 768:N], out_sb[:, 768:N])

            def merge(streams, clocks):
                idx = [0] * len(streams)
                seq = []
                while True:
                    best = None
                    for si, (st, ix) in enumerate(zip(streams, idx)):
                        if ix >= len(st):
                            continue
                        f = max(clocks[si], ready(st[ix])) + cost(st[ix])
                        if best is None or f < best[0]:
                            best = (f, si)
                    if best is None:
                        break
                    f, si = best
                    seq.append(streams[si][idx[si]])
                    clocks[si] = f
                    idx[si] += 1
                return seq, clocks

            clocks = [0.0, 0.0]
            seq1, clocks = merge([dve1, act1], clocks)
            seq2, clocks = merge([dve2, act2], clocks)

            def writes_a(u):
                return u[0] in ("drow", "arow") and u[1] < 64
            last_a = max(i for i, u in enumerate(seq1) if writes_a(u))
            full = seq1 + [("evA", None)] + seq2
            last_b = max(
                i for i, u in enumerate(full) if u[0] in ("drow", "arow", "rect")
            )
            full += [("evB1", None), ("evB2", None)]
            for i, u in enumerate(full):
                flags = {}
                if i == last_a or (i < len(seq1) and i == last_a):
                    flags["stop_a"] = True
                if i == last_b:
                    flags["stop_b"] = True
                emit(u, **flags)

    nc.compile()
    return nc


def _get_program():
    if "nc" not in _PROG_CACHE:
        _PROG_CACHE["nc"] = _build_program()
    return _PROG_CACHE["nc"]


def _install_ntff_hook():
    """The agent image's ``antenv`` lacks ``axon_hooks``, so axon-side NTFF
    profiling silently degrades. Recreate the module and install the ctypes
    hook so trace=True yields exec_time_ns. No-op if unavailable."""
    import sys
    import types

    try:
        import antenv.axon_hooks  # noqa: F401

        return
    except ImportError:
        pass
    try:
        import antenv
        from trn_agent_boot.trn_boot import _ntff_profile_via_ctypes

        mod = types.ModuleType("antenv.axon_hooks")
        mod._hook = _ntff_profile_via_ctypes("/opt/axon/libaxon_pjrt.so")
        mod.set_axon_ntff_profile_hook = lambda h: setattr(mod, "_hook", h)
        mod.get_axon_ntff_profile_hook = lambda: mod._hook
        sys.modules["antenv.axon_hooks"] = mod
        antenv.axon_hooks = mod
    except Exception:
        pass


def kernel(
    next_state_embedding,
    prev_variable_embeddings,
    Wq,
    bq,
    Wk,
    bk,
    Wc,
    bc,
    W1,
    b1,
    W2,
    b2,
):
    from concourse.bass_utils import run_bass_kernel_spmd

    trace = bool(int(os.environ.get("KBENCH_TRACE", "0")))
    if trace:
        _install_ntff_hook()

    nse = np.asarray(next_state_embedding, dtype=np.float32)
    pve = np.asarray(prev_variable_embeddings, dtype=np.float32)
    Wq = np.asarray(Wq, dtype=np.float32)
    bq = np.asarray(bq, dtype=np.float32)
    Wk = np.asarray(Wk, dtype=np.float32)
    bk = np.asarray(bk, dtype=np.float32)
    Wc = np.asarray(Wc, dtype=np.float32)
    bc = np.asarray(bc, dtype=np.float32)
    W1 = np.asarray(W1, dtype=np.float32)
    b1 = np.asarray(b1, dtype=np.float32)
    W2 = np.asarray(W2, dtype=np.float32)
    b2 = np.asarray(b2, dtype=np.float32)

    # host-side projections (tiny)
    context = np.maximum(nse @ Wc + bc, 0.0)
    queries = pve @ Wq + bq + context
    keys = pve @ Wk + bk + context
    q_proj = queries @ W1[:H]  # [N, H]
    k_proj = keys @ W1[H:]  # [N, H]

    kpt = np.ascontiguousarray(k_proj.T, dtype=np.float16)  # [H, N]
    w2w = np.zeros((H, 96), dtype=np.float16)
    w2w[:, 63] = W2[:, 0].astype(np.float16)
    w2f = W2[:, 0].astype(np.float32)

    in_maps = []
    corrs = []
    for d in range(NCORES):
        qb = q_proj[d::NCORES] + b1            # [128, H]
        qbt = np.ascontiguousarray(qb.T, dtype=np.float32)   # [H, 128]
        mqb16 = (-qbt).astype(np.float16)                    # [H, 128]
        mqb2 = np.ascontiguousarray(np.repeat(mqb16, 2, axis=1))  # [H, 256]
        head = np.concatenate(
            [
                qbt.view(np.uint8).reshape(H, 512),
                w2w.view(np.uint8).reshape(H, 192),
                mqb2.view(np.uint8).reshape(H, 512),
            ],
            axis=1,
        )
        head = np.ascontiguousarray(head)
        # mega rows compute max(kpt, mqb); host adds C_k = w2 . (-mqb).
        qb_eff = -mqb16.astype(np.float32)                   # [H, 128]
        corr = w2f @ qb_eff                                  # [128]
        corr[:MEGA0] = 0.0
        corrs.append(corr)
        in_maps.append({"head": head, "kpt": kpt})

    nc = _get_program()
    res = None
    for attempt in range(3):
        try:
            res = run_bass_kernel_spmd(
                nc,
                in_maps,
                core_ids=list(range(NCORES)),
                trace=trace,
            )
            break
        except Exception:
            if attempt == 2:
                raise
            import time

            time.sleep(2.0)
    kernel.last_results = res

    perm = np.array([psum_partition(k) for k in range(NROWS)])
    scores = np.empty((N, N), dtype=np.float32)
    for d in range(NCORES):
        slab = res.results[d]["out"][perm, :].astype(np.float32)  # [128, N]
        slab += corrs[d][:, None]
        scores[d::NCORES, :] = slab

    iu, ju = np.triu_indices(N, k=1)
    return (scores[iu, ju] + b2[0]).astype(np.float32)


kernel.last_results = None


# revision 19
# speedup vs baseline: 1.0125x; 1.0125x over previous
"""Trainium2 Bass kernel for the BackwardVariableSplitter pair scorer.

reference math:
    context = relu(nse @ Wc + bc)                      # [128]
    queries = pve @ Wq + bq + context                  # [1024, 128]
    keys    = pve @ Wk + bk + context                  # [1024, 128]
    q_proj  = queries @ W1[:128]                       # [1024, 128]
    k_proj  = keys @ W1[128:]                          # [1024, 128]
    hidden[i,j] = relu(q_proj[i] + k_proj[j] + b1)     # [1024, 1024, 128]
    scores[i,j] = hidden[i,j] @ W2 + b2                # [1024, 1024]
    out = scores[i, j] for i < j, row-major            # [523776]

The O(n*d*h) projections are tiny and run on the host; the O(n^2*h)
relu + weighted-reduce runs on 8 NeuronCores.

Sharding: core d owns query rows {i : i % 8 == d} (interleaved), so the
triangular (j > i) workload is balanced and the SPMD program is identical
on every core: local row k (global i = 8k + d) computes columns
j in [8k, 1024).

Per-core device program (HW-measured engine costs: DVE tensor_scalar
fp16 4x mode ~129 + 0.26*w ns/instr; ACT activation ~188 + 0.83*w;
DVE tensor_tensor fp16 2x mode ~129 + 0.52*elems/instr):

  - DVE per-row (DVE_ROWS, the wide blocks): X = relu(kpt[:, 8k:] + qb_k)
    via one tensor_scalar (add, max0) at 4x perf mode; the 129ns fixed
    cost amortizes best over wide rows.
  - ACT per-row (ACT_ROWS): same via activation(Relu, bias=qb_k).
  - DVE mega (rows >= 96, rectangles of 8 rows): uses the exact identity
    relu(a+b) = max(a,-b) + b, so a single stock tensor_tensor MAX
    instruction computes X' = max(kpt[:, j0:], -qb_k) for 8 rows at
    once (no per-row instruction cost). The broadcast of -qb_k along j
    is expressed with a duplicated-pair buffer (mqb2[:, 2k] =
    mqb2[:, 2k+1] = -qb[:, k]) and a [k][w/2][pair] access pattern whose
    innermost dim has stride 1 and count 2 - keeping every operand
    eligible for the DVE 2x_1P perf mode (verified on HW: ~0.46-0.52
    ns/elem vs 1.04 at 1x). The missing "+ qb" term is linear, so it
    folds into the host-side per-row constant C_k = w2 . qb_k added
    after the PE reduce. The DVE/ACT/mega row split is the DP optimum
    of max(T_DVE, T_ACT) under the measured cost models.

  Scheduling: units are EMITTED in predicted-completion order (merged
  per-engine streams) because TensorE executes its matmul FIFO in
  program order - the PE stream must match the order X tiles actually
  materialize. Evictions are emitted only after every writer of their
  PSUM region: Tile gives later-emitted matmuls a write-after-read dep
  on an earlier-emitted eviction, which silently drops their
  contribution (found the hard way).
  - TensorE: one-hot W2 stationary window [128, 32] puts w2 in column
    (k//4), so row k's scores accumulate into PSUM partition
    32*(k%4) + k//4. tile_position=(0, 32*(k%4)) spreads consecutive
    rows across the 4 PE column groups (concurrent streams). Banks are
    pre-zeroed by start=True all-zero matmuls.
  - PSUM bank A = columns [0,512), bank B = [512,1024). B is evicted in
    two halves (B1 [512,768) whose writers are all rows < 96, B2
    [768,1024)) so output DMA overlaps the tail; the last-emitted unit
    is the smallest mega rect to minimize the final PE drain.
  - Inputs ship as one 1216B/partition "head" DMA (qb fp32 | w2 window |
    -qb fp16 pairs) on the sync queue plus two kpt chunks (tail columns
    first) on the scalar queue. Few, large DMAs: every extra DMA costs
    ~0.5-1us of queue serialization, and <1KB/partition lines halve DMA
    bandwidth. Output is fp16 (host upcasts); host adds C_k + b2.
"""

import os
import numpy as np

N = 1024
E = 256
H = 128
NCORES = 8
NROWS = N // NCORES  # 128 local rows per core

# engine split (tunable): rows [0, N_D) DVE per-row; [N_D, 96) ACT per-row;
# ACT_WARM rows starting at 96 also ACT (they only need the first kpt chunk,
# so ACT can start before kpt[512:768] lands); the rest mega-TT on DVE.
# DP-optimal block assignment (measured costs): DVE per-row on blocks
# 0-5 and 7, ACT on blocks 6 and 8-11, DVE-mega on blocks 12-15.
DVE_ROWS = list(range(0, 48)) + list(range(56, 65))
ACT_ROWS = list(range(48, 56)) + list(range(65, 96))
MEGA0 = 96
RECT = 8  # rows per mega rectangle

_PROG_CACHE = {}


def psum_partition(k: int) -> int:
    return 32 * (k % 4) + k // 4


def _mega_rects():
    """[(k0, nrows, j0, w0), ...] covering rows [MEGA0, 128)."""
    rects = []
    k = MEGA0
    while k < NROWS:
        nr = min(RECT, NROWS - k)
        j0 = 8 * k
        rects.append((k, nr, j0, N - j0))
        k += nr
    return rects  # [(96,8,768,256), (104,8,832,192), (112,8,896,128), (120,8,960,64)]


def _build_program():
    import concourse.bacc as bacc
    import concourse.tile as tile
    import concourse.mybir as mybir

    nc = bacc.Bacc(
        "TRN2",
        target_bir_lowering=False,
        enable_partition_id=False,
        detect_race_conditions=False,
    )

    fp16 = mybir.dt.float16
    fp32 = mybir.dt.float32
    u8 = mybir.dt.uint8

    HEADB = 512 + 192 + 512  # qbt fp32 | w2w fp16 | mqb2 fp16 (bytes/partition)
    head_d = nc.dram_tensor("head", [H, HEADB], u8, kind="ExternalInput")
    kpt_d = nc.dram_tensor("kpt", [H, N], fp16, kind="ExternalInput")
    out_d = nc.dram_tensor("out", [H, N], fp16, kind="ExternalOutput")

    rects = _mega_rects()

    with tile.TileContext(nc) as tc:
        with (
            tc.tile_pool(name="sb", bufs=1) as sb,
            tc.tile_pool(name="ps", bufs=1, space="PSUM") as ps,
        ):
            head = sb.tile([H, HEADB], u8)
            kpt = sb.tile([H, N], fp16)
            zw = sb.tile([H, 256], fp16)
            out_sb = sb.tile([H, N], fp16)

            qbt = head[:, 0:512].bitcast(fp32)     # [H, 128] fp32 (+qb)
            w2w = head[:, 512:704].bitcast(fp16)   # [H, 96]
            mqb2 = head[:, 704:1216].bitcast(fp16)  # [H, 256] fp16 (-qb pairs)

            # few, big input DMAs (extra DMAs cost ~0.5-1us of queue
            # serialization each); the kpt tail ships first since all the
            # early (narrow) work reads only columns >= 640.
            nc.sync.dma_start(head[:], head_d[:, :])
            nc.scalar.dma_start(kpt[:, 640:N], kpt_d[:, 640:N])
            nc.scalar.dma_start(kpt[:, 0:640], kpt_d[:, 0:640])

            nc.gpsimd.memset(zw[:], 0.0)
            # warm up ACT's Relu table while the DMAs stream
            nc.scalar.activation(
                out_sb[:, 0:16], out_sb[:, 0:16],
                mybir.ActivationFunctionType.Relu,
            )

            psA = ps.tile([H, 512], fp32)  # columns [0, 512)
            psB = ps.tile([H, 512], fp32)  # columns [512, 1024)

            for bank in (psA, psB):
                for half in range(2):
                    nc.tensor.matmul(
                        bank[:, 256 * half: 256 * half + 256],
                        zw[:, 0:H],
                        zw[:],
                        start=True,
                        stop=False,
                        skip_group_check=True,
                    )

            x_dve = {}
            x_act = {}
            x_rect = {}
            for k in DVE_ROWS:
                x_dve[k] = sb.tile([H, N - 8 * k], fp16, name=f"xd{k}")
            for k in ACT_ROWS:
                x_act[k] = sb.tile([H, N - 8 * k], fp16, name=f"xa{k}")
            for (k0, nr, j0, w0) in rects:
                x_rect[k0] = sb.tile([H, nr, w0], fp16, name=f"xr{k0}")

            def emit_mms(k, xap, js, stop_a=False, stop_b=False):
                """PE reduce for local row k whose X tile starts at col js."""
                g = k % 4
                m = k // 4
                lhsT = w2w[:, 63 - m: 95 - m]
                pslice = slice(32 * g, 32 * g + 32)
                if js < 512:
                    wa = 512 - js
                    nc.tensor.matmul(
                        psA[pslice, js:512], lhsT, xap[:, 0:wa],
                        start=False, stop=stop_a, skip_group_check=True,
                        tile_position=(0, 32 * g),
                    )
                    nc.tensor.matmul(
                        psB[pslice, :], lhsT, xap[:, wa: wa + 512],
                        start=False, stop=stop_b, skip_group_check=True,
                        tile_position=(0, 32 * g),
                    )
                else:
                    nc.tensor.matmul(
                        psB[pslice, js - 512: 512], lhsT, xap[:],
                        start=False, stop=stop_b, skip_group_check=True,
                        tile_position=(0, 32 * g),
                    )

            def dve_row(k, stop_a=False, stop_b=False):
                x = x_dve[k][:, :]
                nc.vector.tensor_scalar(
                    x, kpt[:, 8 * k: N], qbt[:, k: k + 1], 0.0,
                    op0=mybir.AluOpType.add, op1=mybir.AluOpType.max,
                )
                emit_mms(k, x, 8 * k, stop_a=stop_a, stop_b=stop_b)

            def act_row(k, stop_a=False, stop_b=False):
                x = x_act[k][:, :]
                nc.scalar.activation(
                    x, kpt[:, 8 * k: N],
                    mybir.ActivationFunctionType.Relu,
                    bias=qbt[:, k: k + 1], scale=1.0,
                )
                emit_mms(k, x, 8 * k, stop_a=stop_a, stop_b=stop_b)

            def mega_rect(rect, stop_a=False, stop_b=False):
                k0, nr, j0, w0 = rect
                xt = x_rect[k0]
                out4 = xt[:, :, :].rearrange("p k (a t) -> p k a t", t=2)
                in0 = (
                    kpt[:, None, j0: j0 + w0]
                    .broadcast_to([H, nr, w0])
                    .rearrange("p k (a t) -> p k a t", t=2)
                )
                in1 = (
                    mqb2[:, 2 * k0: 2 * (k0 + nr)]
                    .rearrange("p (k t) -> p k t", t=2)[:, :, None, :]
                    .broadcast_to([H, nr, w0 // 2, 2])
                )
                nc.vector.tensor_max(out4, in0, in1)
                for t in range(nr):
                    k = k0 + t
                    emit_mms(
                        k, xt[:, t, :], j0,
                        stop_b=(stop_b and t == nr - 1),
                    )

            # ---- schedule ----
            # Per-engine unit order is chosen for data arrival (narrow work
            # first: it only needs the kpt tail + head DMAs) and for early
            # PSUM-region completion. Units from the two engines are EMITTED
            # in predicted-completion order so the TensorE FIFO sees matmuls
            # in the order their rhs tiles actually materialize.
            def cost(unit):
                kind, arg = unit
                if kind == "rect":
                    k0, nr, j0, w0 = arg
                    return 129 + 0.521 * nr * w0
                if kind == "drow":
                    return 129 + 0.26 * (N - 8 * arg)
                if kind == "arow":
                    return 186 + 0.833 * (N - 8 * arg)
                return 500.0  # evictions

            def ready(unit):
                kind, arg = unit
                if kind == "drow":
                    return 1300.0  # kpt[0:512] chunk
                if kind == "arow" and arg < 64:
                    return 1300.0
                if kind == "arow":
                    return 600.0  # kpt[512:1024] chunk
                return 0.0

            # stage 1 contains every psA and psB1 writer; stage 2 is the
            # pure-B2 mega rect. Narrow work leads (it only needs the kpt
            # tail chunk + head DMAs).
            dve1 = (
                [("rect", rects[2]), ("rect", rects[1]), ("rect", rects[0])]
                + [("drow", k) for k in DVE_ROWS]
            )
            dve2 = [("rect", rects[3])]
            act1 = (
                [("arow", k) for k in range(95, 87, -1)]
                + [("arow", k) for k in range(48, 56)]
                + [("arow", k) for k in range(87, 64, -1)]
            )
            act2 = []

            def emit(unit, stop_a=False, stop_b=False):
                kind, arg = unit
                if kind == "rect":
                    mega_rect(arg, stop_a=stop_a, stop_b=stop_b)
                elif kind == "drow":
                    dve_row(arg, stop_a=stop_a, stop_b=stop_b)
                elif kind == "arow":
                    act_row(arg, stop_a=stop_a, stop_b=stop_b)
                elif kind == "evA":
                    nc.vector.tensor_copy(out_sb[:, 0:512], psA[:])
                    nc.sync.dma_start(out_d[:, 0:512], out_sb[:, 0:512])
                elif kind == "evB1":
                    nc.scalar.copy(out_sb[:, 512:768], psB[:, 0:256])
                    nc.scalar.dma_start(out_d[:, 512:768], out_sb[:, 512:768])
                elif kind == "evB2":
                    nc.vector.tensor_copy(out_sb[:, 768:N], psB[:, 256:512])
                    nc.sync.dma_start(out_d[:, 768:N], out_sb[:, 768:N])

            def merge(streams, clocks):
                idx = [0] * len(streams)
                seq = []
                while True:
                    best = None
                    for si, (st, ix) in enumerate(zip(streams, idx)):
                        if ix >= len(st):
                            continue
                        f = max(clocks[si], ready(st[ix])) + cost(st[ix])
                        if best is None or f < best[0]:
                            best = (f, si)
                    if best is None:
                        break
                    f, si = best
                    seq.append(streams[si][idx[si]])
                    clocks[si] = f
                    idx[si] += 1
                return seq, clocks

            clocks = [0.0, 0.0]
            seq1, clocks = merge([dve1, act1], clocks)
            seq2, clocks = merge([dve2, act2], clocks)

            def writes_a(u):
                return u[0] in ("drow", "arow") and u[1] < 64
            last_a = max(i for i, u in enumerate(seq1) if writes_a(u))
            full = seq1 + [("evA", None)] + seq2
            last_b = max(
                i for i, u in enumerate(full) if u[0] in ("drow", "arow", "rect")
            )
            full += [("evB1", None), ("evB2", None)]
            for i, u in enumerate(full):
                flags = {}
                if i == last_a or (i < len(seq1) and i == last_a):
                    flags["stop_a"] = True
                if i == last_b:
                    flags["stop_b"] = True
                emit(u, **flags)

    nc.compile()
    return nc


def _get_program():
    if "nc" not in _PROG_CACHE:
        _PROG_CACHE["nc"] = _build_program()
    return _PROG_CACHE["nc"]


def _install_ntff_hook():
    """The agent image's ``antenv`` lacks ``axon_hooks``, so axon-side NTFF
    profiling silently degrades. Recreate the module and install the ctypes
    hook so trace=True yields exec_time_ns. No-op if unavailable."""
    import sys
    import types

    try:
        import antenv.axon_hooks  # noqa: F401

        return
    except ImportError:
        pass
    try:
        import antenv
        from trn_agent_boot.trn_boot import _ntff_profile_via_ctypes

        mod = types.ModuleType("antenv.axon_hooks")
        mod._hook = _ntff_profile_via_ctypes("/opt/axon/libaxon_pjrt.so")
        mod.set_axon_ntff_profile_hook = lambda h: setattr(mod, "_hook", h)
        mod.get_axon_ntff_profile_hook = lambda: mod._hook
        sys.modules["antenv.axon_hooks"] = mod
        antenv.axon_hooks = mod
    except Exception:
        pass


def kernel(
    next_state_embedding,
    prev_variable_embeddings,
    Wq,
    bq,
    Wk,
    bk,
    Wc,
    bc,
    W1,
    b1,
    W2,
    b2,
):
    from concourse.bass_utils import run_bass_kernel_spmd

    trace = bool(int(os.environ.get("KBENCH_TRACE", "0")))
    if trace:
        _install_ntff_hook()

    nse = np.asarray(next_state_embedding, dtype=np.float32)
    pve = np.asarray(prev_variable_embeddings, dtype=np.float32)
    Wq = np.asarray(Wq, dtype=np.float32)
    bq = np.asarray(bq, dtype=np.float32)
    Wk = np.asarray(Wk, dtype=np.float32)
    bk = np.asarray(bk, dtype=np.float32)
    Wc = np.asarray(Wc, dtype=np.float32)
    bc = np.asarray(bc, dtype=np.float32)
    W1 = np.asarray(W1, dtype=np.float32)
    b1 = np.asarray(b1, dtype=np.float32)
    W2 = np.asarray(W2, dtype=np.float32)
    b2 = np.asarray(b2, dtype=np.float32)

    # host-side projections (tiny)
    context = np.maximum(nse @ Wc + bc, 0.0)
    queries = pve @ Wq + bq + context
    keys = pve @ Wk + bk + context
    q_proj = queries @ W1[:H]  # [N, H]
    k_proj = keys @ W1[H:]  # [N, H]

    kpt = np.ascontiguousarray(k_proj.T, dtype=np.float16)  # [H, N]
    w2w = np.zeros((H, 96), dtype=np.float16)
    w2w[:, 63] = W2[:, 0].astype(np.float16)
    w2f = W2[:, 0].astype(np.float32)

    in_maps = []
    corrs = []
    for d in range(NCORES):
        qb = q_proj[d::NCORES] + b1            # [128, H]
        qbt = np.ascontiguousarray(qb.T, dtype=np.float32)   # [H, 128]
        mqb16 = (-qbt).astype(np.float16)                    # [H, 128]
        mqb2 = np.ascontiguousarray(np.repeat(mqb16, 2, axis=1))  # [H, 256]
        head = np.concatenate(
            [
                qbt.view(np.uint8).reshape(H, 512),
                w2w.view(np.uint8).reshape(H, 192),
                mqb2.view(np.uint8).reshape(H, 512),
            ],
            axis=1,
        )
        head = np.ascontiguousarray(head)
        # mega rows compute max(kpt, mqb); host adds C_k = w2 . (-mqb).
        qb_eff = -mqb16.astype(np.float32)                   # [H, 128]
        corr = w2f @ qb_eff                                  # [128]
        corr[:MEGA0] = 0.0
        corrs.append(corr)
        in_maps.append({"head": head, "kpt": kpt})

    nc = _get_program()
    res = None
    for attempt in range(3):
        try:
            res = run_bass_kernel_spmd(
                nc,
                in_maps,
                core_ids=list(range(NCORES)),
                trace=trace,
            )
            break
        except Exception:
            if attempt == 2:
                raise
            import time

            time.sleep(2.0)
    kernel.last_results = res

    perm = np.array([psum_partition(k) for k in range(NROWS)])
    scores = np.empty((N, N), dtype=np.float32)
    for d in range(NCORES):
        slab = res.results[d]["out"][perm, :].astype(np.float32)  # [128, N]
        slab += corrs[d][:, None]
        scores[d::NCORES, :] = slab

    iu, ju = np.triu_indices(N, k=1)
    return (scores[iu, ju] + b2[0]).astype(np.float32)


kernel.last_results = None
